# revision 20
# baseline (speedup 1.0000x reference)
"""Trainium2 Bass kernel for nn_Convolution (e3nn-style GNN message passing).

Strategy (8 NeuronCores, SPMD, no collectives):
- Sort edges by destination; core c owns destination nodes [6400c, 6400(c+1)).
- Per core: edges binned into 50 node-blocks (128 nodes each), padded to NG
  groups of 128 edges. Dummy edges gather a zero table row -> contribute 0.
- Gather source features with dma_gather (bf16 table, 256B rows, 1024 idx per
  call = 2 windows) on gpsimd; this is the Q7-descgen-bound roof (~5-7ns/idx).
- Radial MLP layer 1 on PE (bf16, tile_position row-packed K=8), relu on ACT,
  layer 2 on PE (bf16, h stationary); w copied PSUM->SBUF bf16 on ACT.
- TP products on DVE in bf16 with (w,u)-transposed per-edge weights so the
  big products hit 2x/4x DVE modes. Path0/1 and path3 keep the u-contraction
  deferred into the scatter matmul (per-block PSUM accumulation); path2 is
  contracted early (scat width 128+24+192 = 344).
- Scatter via host-baked one-hot (bf16, DMA-streamed) matmuls into PSUM per
  node-block; per-block u-reduction on DVE, then DMA out.
"""

import math
import os
import numpy as np
import ml_dtypes

_TRACE_SIM = bool(int(os.environ.get('K_TRACE_SIM', '0')))

import concourse.bass as bass
import concourse.bacc as bacc
import concourse.mybir as mybir
from concourse.tile import TileContext
from concourse.bass_utils import run_bass_kernel_spmd

# ---------------- problem constants (hardcoded per spec) ----------------
N_NODES, N_EDGES, NUM_BASIS, HIDDEN = 50000, 800000, 8, 256
MUL = 8
INV_SQRT3 = float(1.0 / np.sqrt(3.0))
A_SCALAR = float(np.sqrt(1.0 / 128.0))
A_VECTOR = float(np.sqrt(3.0 / 128.0))
SQRT2 = float(np.sqrt(2.0))
DEG_SCALE = float(1.0 / np.sqrt(N_EDGES / N_NODES))

NCORES = 8
P = 128
NODES_PER_CORE = 6400          # 50 blocks of 128; 8*6400 = 51200 >= 50000
NB = 50                        # node blocks per core
# table rows 1..50000 = nodes 0..49999; row 50001 = zeros (dummy target).
# int16 gather idx = node + 1 - GBASE; dummy idx = +17233 (>= 0, never hits
# the trailing-negative trim). Host swaps an in-block edge so the last
# (trim-order) index of each gather call is >= 0.
TBL_ROWS = 50004
GBASE = 32768
DUMMY_IDX = 50001 - GBASE
SUPER_G = 8                    # groups per gather call (1024 idx)
BF16 = ml_dtypes.bfloat16

_PROG_CACHE = {}


# ---------------- device program ----------------
def _build_program(NG):
    GROUPS = NB * NG
    WINDOWS = GROUPS // 4
    NSUPER = (GROUPS + SUPER_G - 1) // SUPER_G
    IDXW = SUPER_G * 128 // 16       # wrapped idx cols per (full) super
    NJ = (WINDOWS + 3) // 4          # es_w4 column blocks
    ES_CHUNK_J = 3
    OH_CHUNK = 2                     # windows of one-hot per DMA

    nc = bacc.Bacc(num_devices=NCORES, num_swdge_queues=4)
    f32, i16, bf = mybir.dt.float32, mybir.dt.int16, mybir.dt.bfloat16

    tbl = nc.dram_tensor("tbl", [TBL_ROWS, 128], bf, kind="ExternalInput")
    idx_g = nc.dram_tensor("idx_g", [P, NSUPER * IDXW], i16, kind="ExternalInput")
    es4 = nc.dram_tensor("es4", [P, NJ * 512], bf, kind="ExternalInput")
    sh_t = nc.dram_tensor("sh_t", [P, GROUPS * 4], bf, kind="ExternalInput")
    oh_t = nc.dram_tensor("oh_t", [P, GROUPS * 128], bf, kind="ExternalInput")
    w1t = nc.dram_tensor("w1t", [P, 256], bf, kind="ExternalInput")
    w2t = nc.dram_tensor("w2t", [P, 512], bf, kind="ExternalInput")
    nodeout = nc.dram_tensor("nodeout", [NODES_PER_CORE, 32], f32, kind="ExternalOutput")

    AX = mybir.AxisListType.X
    ADD = mybir.AluOpType.add
    MUL_ = mybir.AluOpType.mult
    RELU = mybir.ActivationFunctionType.Relu
    COPY = mybir.ActivationFunctionType.Copy

    with TileContext(nc, trace_sim=_TRACE_SIM) as tc:
        with tc.tile_pool(name="const", bufs=1) as cpool, \
             tc.tile_pool(name="stream", bufs=3) as spool, \
             tc.tile_pool(name="xsp", bufs=4) as xspool, \
             tc.tile_pool(name="work", bufs=3) as wpool, \
             tc.tile_pool(name="psum", bufs=2, space="PSUM") as pp, \
             tc.tile_pool(name="psum1", bufs=1, space="PSUM") as pp1:

            # constants resident in SBUF
            ig_sb = cpool.tile([P, NSUPER * IDXW], i16, name="ig")
            nc.sync.dma_start(ig_sb[:], idx_g[:])
            sh_sb = cpool.tile([P, GROUPS, 4], bf, name="sh")
            nc.sync.dma_start(sh_sb[:], sh_t[:].rearrange("p (g k) -> p g k", k=4))
            w1_sb = cpool.tile([P, 256], bf, name="w1")
            nc.sync.dma_start(w1_sb[:], w1t[:])
            w2_sb = cpool.tile([P, 2, 256], bf, name="w2")
            nc.sync.dma_start(w2_sb[:], w2t[:].rearrange("p (h n) -> p h n", h=2))

            # pre-zero the xs ring: trimmed (padded) gather slots keep stale
            # SBUF content, which must be finite (0 * NaN would poison PSUM)
            for _ in range(4):
                xs_z = xspool.tile([P, SUPER_G, 128], bf, tag="xs")
                nc.vector.memset(xs_z[:], 0.0)

            acc_box = [None]
            xs_sup = None

            def phase_b(st):
                """Products + scatter + block-end for a finished window."""
                g0 = st["g0"]
                shw, xs, w_sb = st["shw"], st["xs"], st["w_sb"]
                ab16, v1s2 = st["ab16"], st["v1s2"]
                oh_sb, ohw = st["oh_sb"], st["ohw"]
                scat = wpool.tile([P, 4, 344], bf, tag="scat")
                # path0/1 (deferred u): scat[:, 0:128] (w,u') = ab16[u']*w01T[w,u']
                nc.vector.tensor_tensor(
                    out=scat[:, :, 0:128].rearrange("p g (w u) -> p g w u", w=8),
                    in0=ab16[:].unsqueeze(2).to_broadcast([P, 4, 8, 16]),
                    in1=w_sb[:, :, 0:128].rearrange("p g (w u) -> p g w u", w=8),
                    op=MUL_,
                )
                # path2 (early u): t2[w] = sum_u s1[u] * w2T[w,u]
                t2p = wpool.tile([P, 4, 8, 8], bf, tag="t2p")
                nc.vector.tensor_tensor(
                    out=t2p[:],
                    in0=xs[:, :, 0:8].unsqueeze(2).to_broadcast([P, 4, 8, 8]),
                    in1=w_sb[:, :, 128:192].rearrange("p g (w u) -> p g w u", w=8),
                    op=MUL_,
                )
                t2 = wpool.tile([P, 4, 8], bf, tag="t2")
                with nc.allow_low_precision("t2 is an 8-term dot, bf16 ok"):
                    nc.vector.tensor_reduce(out=t2[:], in_=t2p[:], axis=AX, op=ADD)
                # scat[:, 128:152] (w, i) = t2[w] * v2[i]
                nc.vector.tensor_tensor(
                    out=scat[:, :, 128:152].rearrange("p g (w i) -> p g w i", w=8),
                    in0=t2[:].unsqueeze(3).to_broadcast([P, 4, 8, 3]),
                    in1=shw[:, :, 1:4].unsqueeze(2).to_broadcast([P, 4, 8, 3]),
                    op=MUL_,
                )
                # path3 (deferred u): scat[:, 152:344] (i,w,u) = v1s2T[i,u]*w3T[w,u]
                # one op per i (DVE ISA allows at most 3 free AP dims)
                for i3 in range(3):
                    nc.vector.tensor_tensor(
                        out=scat[:, :, 152 + 64 * i3 : 216 + 64 * i3].rearrange(
                            "p g (w u) -> p g w u", w=8
                        ),
                        in0=v1s2[:, :, i3, :].unsqueeze(2).to_broadcast([P, 4, 8, 8]),
                        in1=w_sb[:, :, 192:256].rearrange(
                            "p g (w u) -> p g w u", w=8
                        ),
                        op=MUL_,
                    )

                # --- per group: scatter matmul into block accumulator
                for gg in range(4):
                    g = g0 + gg
                    b = g // NG
                    gib = g % NG
                    if gib == 0:
                        acc_new = pp.tile([P, 344], f32, space="PSUM", tag="acc")
                        acc_box[0] = acc_new
                    acc_ps = acc_box[0]
                    nc.tensor.matmul(
                        acc_ps[:],
                        lhsT=oh_sb[:, ohw * 4 + gg, :], rhs=scat[:, gg, :],
                        start=(gib == 0), stop=(gib == NG - 1),
                    )
                    if gib == NG - 1:
                        stage = wpool.tile([P, 32], f32, tag="stage")
                        nc.vector.tensor_reduce(
                            out=stage[:, 0:8],
                            in_=acc_ps[:, 0:128].rearrange(
                                "p (w u) -> p w u", w=8
                            ),
                            axis=AX, op=ADD,
                        )
                        t24 = wpool.tile([P, 24], f32, tag="t24")
                        nc.vector.tensor_reduce(
                            out=t24[:],
                            in_=acc_ps[:, 152:344].rearrange(
                                "p (iw u) -> p iw u", u=8
                            ),
                            axis=AX, op=ADD,
                        )
                        # t24 is (i, w); stage[:, 8:32] wants (w, i)
                        nc.vector.tensor_tensor(
                            out=stage[:, 8:32].rearrange("p (w i) -> p w i", w=8),
                            in0=t24[:].rearrange("p (i w) -> p w i", i=3),
                            in1=acc_ps[:, 128:152].rearrange(
                                "p (w i) -> p w i", w=8
                            ),
                            op=ADD,
                        )
                        nc.sync.dma_start(
                            nodeout[128 * b : 128 * b + 128, :], stage[:]
                        )

            # 3-stage software pipeline:
            #   iter w: MLP1(w)+relu(w)+smalls(w) | MLP2(w-1) | wcopy(w-2)
            #           +prods(w-2)+scatter(w-2)+blockend(w-2)
            st = {}          # per-window deferred state
            for w in range(WINDOWS + 2):
                if w < WINDOWS:
                    c = w % 4
                    j = w // 4
                    g0 = 4 * w

                    # --- stream es chunk (every ES_CHUNK_J col-blocks)
                    if j % ES_CHUNK_J == 0 and c == 0:
                        jw = min(ES_CHUNK_J, NJ - j)
                        es_sb = spool.tile([P, ES_CHUNK_J * 512], bf, tag="es")
                        nc.sync.dma_start(
                            es_sb[:, : jw * 512], es4[:, j * 512 : (j + jw) * 512]
                        )
                    jj = j % ES_CHUNK_J

                    # --- stream one-hot (every OH_CHUNK windows)
                    if w % OH_CHUNK == 0:
                        nwoh = min(OH_CHUNK, WINDOWS - w)
                        oh_sb = spool.tile([P, OH_CHUNK * 4, P], bf, tag="oh")
                        nc.sync.dma_start(
                            oh_sb[:, : nwoh * 4, :],
                            oh_t[:, g0 * 128 : (g0 + nwoh * 4) * 128].rearrange(
                                "p (g n) -> p g n", n=P
                            ),
                        )
                    ohw = w % OH_CHUNK

                    # --- gather (1024 idx = 2 windows per call)
                    if w % 2 == 0:
                        s = w // 2
                        nidx = min(SUPER_G * 128, (GROUPS - s * SUPER_G) * 128)
                        xs_sup = xspool.tile([P, SUPER_G, 128], bf, tag="xs")
                        nc.gpsimd.dma_gather(
                            out_ap=xs_sup[:, : nidx // 128, :],
                            in_ap=tbl[GBASE:, :],
                            idxs_ap=ig_sb[:, s * IDXW : s * IDXW + nidx // 16],
                            num_idxs=nidx, num_idxs_reg=nidx, elem_size=128,
                            queue_num=s % 4,
                        )
                    xs = xs_sup[:, 4 * (w % 2) : 4 * (w % 2) + 4, 0:32]

                    # --- MLP1: h[comp, edge], two 128-comp halves
                    h_ps = pp1.tile([P, 2, 512], f32, space="PSUM", tag="hps")
                    for half in range(2):
                        nc.tensor.matmul(
                            h_ps[:, half, :],
                            lhsT=w1_sb[32 * c : 32 * c + 8,
                                       half * 128 : half * 128 + 128],
                            rhs=es_sb[32 * c : 32 * c + 8,
                                      jj * 512 : jj * 512 + 512],
                            start=True, stop=True,
                            tile_position=(32 * c, 0),
                        )
                    h_sb = wpool.tile([P, 2, 512], bf, tag="hsb")
                    for half in range(2):
                        nc.scalar.activation(
                            out=h_sb[:, half, :], in_=h_ps[:, half, :], func=RELU
                        )

                    # --- per-window DVE smalls (xs/sh only)
                    shw = sh_sb[:, g0 : g0 + 4, :]        # [P, 4, 4]
                    ab16 = wpool.tile([P, 4, 16], bf, tag="ab16")
                    nc.vector.tensor_tensor(
                        out=ab16[:, :, 0:8], in0=xs[:, :, 0:8],
                        in1=shw[:, :, 0:1].to_broadcast([P, 4, 8]), op=MUL_,
                    )
                    pb = wpool.tile([P, 4, 8, 3], bf, tag="pb")
                    nc.vector.tensor_tensor(
                        out=pb[:],
                        in0=xs[:, :, 8:32].rearrange("p g (u i) -> p g u i", u=8),
                        in1=shw[:, :, 1:4].unsqueeze(2).to_broadcast([P, 4, 8, 3]),
                        op=MUL_,
                    )
                    with nc.allow_low_precision("b-dot is 3 terms, bf16 ok"):
                        nc.vector.tensor_reduce(
                            out=ab16[:, :, 8:16], in_=pb[:], axis=AX, op=ADD
                        )
                    v1s2 = wpool.tile([P, 4, 3, 8], bf, tag="v1s2")
                    nc.vector.tensor_tensor(
                        out=v1s2[:],
                        in0=xs[:, :, 8:32].rearrange("p g (u i) -> p g i u", u=8),
                        in1=shw[:, :, 0:1].unsqueeze(2).to_broadcast([P, 4, 3, 8]),
                        op=MUL_,
                    )
                    st[w] = dict(
                        g0=g0, shw=shw, xs=xs, h_sb=h_sb,
                        ab16=ab16, v1s2=v1s2, oh_sb=oh_sb, ohw=ohw,
                    )

                # --- MLP2 for window w-1
                if 0 <= w - 1 < WINDOWS:
                    s1 = st[w - 1]
                    w_ps = pp.tile([P, 4, 256], f32, space="PSUM", tag="wps")
                    for gg in range(4):
                        for half in range(2):
                            nc.tensor.matmul(
                                w_ps[:, gg, :],
                                lhsT=s1["h_sb"][:, half, gg * 128 : gg * 128 + 128],
                                rhs=w2_sb[:, half, :],
                                start=(half == 0), stop=(half == 1),
                            )
                    s1["w_ps"] = w_ps

                # --- wcopy + products + scatter for window w-2
                if w - 2 >= 0:
                    s2 = st[w - 2]
                    w_sb = wpool.tile([P, 4, 256], bf, tag="wsb")
                    nc.scalar.activation(out=w_sb[:], in_=s2["w_ps"][:], func=COPY)
                    s2["w_sb"] = w_sb
                    phase_b(s2)
                    del st[w - 2]
    nc.compile()
    return nc


# ---------------- host-side prep ----------------
def _prep(node_features, edge_src, edge_dst, edge_sh, edge_scalars, fc_w1, fc_w2,
          NG, perm):
    GROUPS = NB * NG
    EPAD = GROUPS * 128
    WINDOWS = GROUPS // 4
    NSUPER = (GROUPS + SUPER_G - 1) // SUPER_G
    IDXW = SUPER_G * 128 // 16
    NJ = (WINDOWS + 3) // 4

    # fold all scalar coefficients into the weights
    w1s = (fc_w1 * (1.0 / math.sqrt(NUM_BASIS))).astype(np.float32)     # [8, 256]
    w2 = (fc_w2 * (SQRT2 / math.sqrt(HIDDEN))).astype(np.float64)       # [256, 256]
    w2 = w2.reshape(HIDDEN, 4, MUL, MUL)
    coef = np.array(
        [A_SCALAR, A_SCALAR * INV_SQRT3, A_VECTOR * INV_SQRT3, A_VECTOR * INV_SQRT3]
    ) * DEG_SCALE
    w2 = w2 * coef[None, :, None, None]
    # device col order, all (w, u) transposed:
    #   [0:128]  w01T: col w*16+u'  (u'<8: path0, u'>=8: path1)
    #   [128:192] w2T: col 128+w*8+u (path2)
    #   [192:256] w3T: col 192+w*8+u (path3)
    w2dev = np.zeros((HIDDEN, 256), np.float64)
    w2dev[:, 0:128] = np.concatenate(
        [w2[:, 0], w2[:, 1]], axis=1  # [256, u8, w8] x2 -> (u', w)
    ).transpose(0, 2, 1).reshape(HIDDEN, 128)          # (w, u')
    w2dev[:, 128:192] = w2[:, 2].transpose(0, 2, 1).reshape(HIDDEN, 64)  # (w, u)
    w2dev[:, 192:256] = w2[:, 3].transpose(0, 2, 1).reshape(HIDDEN, 64)  # (w, u)
    w2dev = w2dev.astype(np.float32)

    w1t = np.zeros((P, 256), np.float32)
    for c in range(4):
        w1t[32 * c : 32 * c + 8] = w1s
    w2t = np.zeros((P, 512), np.float32)
    w2t[:, 0:256] = w2dev[0:128]
    w2t[:, 256:512] = w2dev[128:256]

    tbl = np.zeros((TBL_ROWS, 128), BF16)
    tbl[1 : N_NODES + 1, 0:32] = node_features.astype(BF16)

    src_all = np.asarray(edge_src).astype(np.int64)
    dst_all = np.asarray(edge_dst).astype(np.int64)
    es_all = np.asarray(edge_scalars).astype(np.float32)
    sh_all = np.asarray(edge_sh).astype(np.float32)
    gdst = perm[dst_all]                      # balanced destination slots
    core_of = gdst // NODES_PER_CORE

    nrange = np.arange(P, dtype=np.int64)
    in_maps = []
    for cid in range(NCORES):
        sel = np.nonzero(core_of == cid)[0]
        loc = gdst[sel] - NODES_PER_CORE * cid
        order = np.argsort(loc >> 7, kind="stable")
        sel = sel[order]
        loc = loc[order]
        blk = loc >> 7
        cnt = np.bincount(blk, minlength=NB)
        assert cnt.max() <= NG * 128, (cid, cnt.max())
        start = np.zeros(NB, np.int64)
        start[1:] = np.cumsum(cnt)[:-1]
        rank = np.arange(len(sel)) - start[blk]
        slot = blk * (NG * 128) + rank

        srcv = np.full(EPAD, -1, np.int64)
        srcv[slot] = src_all[sel]
        shv = np.zeros((EPAD, 4), np.float32)
        shv[slot] = sh_all[sel]
        esv = np.zeros((EPAD, 8), np.float32)
        esv[slot] = es_all[sel]
        dlv = np.full(EPAD, -1, np.int64)
        dlv[slot] = loc & 127

        # --- gather indices: row = node+1, idx = node + 1 - GBASE.
        # Padded slots get idx = -32768 (outside the real range): they sit at
        # each block's tail, which is also a gather-call tail (block = 2
        # supers at NG=16), so the HW trailing-negative trim skips them.
        # Force the last real (trim-order) index of each call >= 0 by
        # swapping with a same-block edge (real negatives would get eaten).
        idxv = np.where(srcv >= 0, srcv + 1 - GBASE, -32768).astype(np.int64)
        BLKE = NG * 128
        SUP = SUPER_G * 128
        # blocks so underfull that a whole gather call would be empty: use
        # gathered dummy-row padding instead of the trim
        for b0 in np.nonzero(cnt < BLKE - SUP + 1)[0]:
            seg = idxv[b0 * BLKE : (b0 + 1) * BLKE]
            seg[seg == -32768] = DUMMY_IDX
        for sgi in range(NSUPER):
            lo, hi = sgi * SUP, min((sgi + 1) * SUP, EPAD)
            real = np.nonzero(idxv[lo:hi] > -32768)[0]
            if len(real) == 0:
                continue
            jl = lo + real[-1]          # last real slot of this call
            if idxv[jl] >= 0:
                continue
            b0 = jl // BLKE
            cand = np.nonzero(idxv[b0 * BLKE : (b0 + 1) * BLKE] >= 0)[0]
            cand = [b0 * BLKE + q for q in cand
                    if b0 * BLKE + q < jl and (b0 * BLKE + q + 1) % SUP != 0]
            if not cand:
                # no positive idx before jl in this block: give up on trim
                # for this block's pads (use the gathered dummy row instead)
                idxv[b0 * BLKE : (b0 + 1) * BLKE][
                    idxv[b0 * BLKE : (b0 + 1) * BLKE] == -32768
                ] = DUMMY_IDX
                continue
            q = cand[-1]
            for arr in (idxv, srcv, dlv):
                arr[jl], arr[q] = arr[q], arr[jl]
            for arr in (shv, esv):
                tmpq = arr[q].copy()
                arr[q] = arr[jl]
                arr[jl] = tmpq
        idx_pad = np.zeros(NSUPER * SUPER_G * 128, np.int64)
        idx_pad[:EPAD] = idxv
        idx_g = np.tile(
            idx_pad.reshape(-1, 16).T.astype(np.int16), (8, 1)
        )  # [128, NSUPER*IDXW]

        # es4: window w at rows 32*(w%4)+b, cols [ (w//4)*512, +512 )
        es4 = np.zeros((P, NJ * 512), np.float32)
        esw = esv.reshape(WINDOWS, 512, 8)
        for c in range(4):
            wsel = np.arange(c, WINDOWS, 4)
            nw = len(wsel)
            es4[32 * c : 32 * c + 8, : nw * 512] = (
                esw[wsel].transpose(2, 0, 1).reshape(8, nw * 512)
            )

        sh_t = shv.reshape(GROUPS, P, 4).transpose(1, 0, 2).reshape(P, GROUPS * 4)
        # one-hot: oh_t[p, g*128 + n] = (dl[g*128+p] == n)
        dlg = dlv.reshape(GROUPS, P)                      # [g, p]
        oh = (dlg[:, :, None] == nrange[None, None, :])   # [g, p, n]
        oh_t = np.ascontiguousarray(
            oh.transpose(1, 0, 2).reshape(P, GROUPS * 128)
        ).astype(BF16)

        in_maps.append(
            dict(
                tbl=tbl, idx_g=np.ascontiguousarray(idx_g),
                es4=np.ascontiguousarray(es4.astype(BF16)),
                sh_t=np.ascontiguousarray(sh_t.astype(BF16)),
                oh_t=oh_t,
                w1t=w1t.astype(BF16), w2t=w2t.astype(BF16),
            )
        )
    return in_maps


def _plan(edge_dst):
    """Degree-balanced node -> global-slot permutation (snake deal over the
    400 (core, block) bins) and the resulting NG."""
    dst_all = np.asarray(edge_dst).astype(np.int64)
    NBINS = NCORES * NB
    TOT = NBINS * P
    degp = np.zeros(TOT, np.int64)
    degp[:N_NODES] = np.bincount(dst_all, minlength=N_NODES)
    order = np.argsort(-degp, kind="stable")
    i = np.arange(TOT)
    r = i // NBINS                            # deal round = slot within bin
    pos = i % NBINS
    binid = np.where(r % 2 == 0, pos, NBINS - 1 - pos)
    perm = np.empty(TOT, np.int64)
    perm[order] = binid * P + r
    load = np.bincount(binid, weights=degp[order].astype(np.float64),
                       minlength=NBINS).astype(np.int64)
    NG = int(math.ceil(load.max() / 128.0))
    if NG % 2:
        NG += 1
    return max(NG, 2), perm


def kernel(node_features, edge_src, edge_dst, edge_sh, edge_scalars, fc_w1, fc_w2):
    node_features = np.asarray(node_features, dtype=np.float32)
    edge_sh = np.asarray(edge_sh, dtype=np.float32)
    edge_scalars = np.asarray(edge_scalars, dtype=np.float32)
    fc_w1 = np.asarray(fc_w1, dtype=np.float32)
    fc_w2 = np.asarray(fc_w2, dtype=np.float32)

    NG, perm = _plan(edge_dst)
    if NG not in _PROG_CACHE:
        _PROG_CACHE[NG] = _build_program(NG)
    nc = _PROG_CACHE[NG]

    in_maps = _prep(
        node_features, edge_src, edge_dst, edge_sh, edge_scalars, fc_w1, fc_w2,
        NG, perm,
    )
    res = run_bass_kernel_spmd(nc, in_maps, core_ids=list(range(NCORES)))
    out = np.concatenate([res.results[c]["nodeout"] for c in range(NCORES)], axis=0)
    return out[perm[:N_NODES]].astype(np.float32)


# revision 21
# speedup vs baseline: 1.0208x; 1.0208x over previous
"""Trainium2 Bass kernel for nn_Convolution (e3nn-style GNN message passing).

Strategy (8 NeuronCores, SPMD, no collectives):
- Sort edges by destination; core c owns destination nodes [6400c, 6400(c+1)).
- Per core: edges binned into 50 node-blocks (128 nodes each), padded to NG
  groups of 128 edges. Dummy edges gather a zero table row -> contribute 0.
- Gather source features with dma_gather (bf16 table, 256B rows, 1024 idx per
  call = 2 windows) on gpsimd; this is the Q7-descgen-bound roof (~5-7ns/idx).
- Radial MLP layer 1 on PE (bf16, tile_position row-packed K=8), relu on ACT,
  layer 2 on PE (bf16, h stationary); w copied PSUM->SBUF bf16 on ACT.
- TP products on DVE in bf16 with (w,u)-transposed per-edge weights so the
  big products hit 2x/4x DVE modes. Path0/1 and path3 keep the u-contraction
  deferred into the scatter matmul (per-block PSUM accumulation); path2 is
  contracted early (scat width 128+24+192 = 344).
- Scatter via host-baked one-hot (bf16, DMA-streamed) matmuls into PSUM per
  node-block; per-block u-reduction on DVE, then DMA out.
"""

import math
import os
import numpy as np
import ml_dtypes

_TRACE_SIM = bool(int(os.environ.get('K_TRACE_SIM', '0')))

import concourse.bass as bass
import concourse.bacc as bacc
import concourse.mybir as mybir
from concourse.tile import TileContext
from concourse.bass_utils import run_bass_kernel_spmd

# ---------------- problem constants (hardcoded per spec) ----------------
N_NODES, N_EDGES, NUM_BASIS, HIDDEN = 50000, 800000, 8, 256
MUL = 8
INV_SQRT3 = float(1.0 / np.sqrt(3.0))
A_SCALAR = float(np.sqrt(1.0 / 128.0))
A_VECTOR = float(np.sqrt(3.0 / 128.0))
SQRT2 = float(np.sqrt(2.0))
DEG_SCALE = float(1.0 / np.sqrt(N_EDGES / N_NODES))

NCORES = 8
P = 128
NODES_PER_CORE = 6400          # 50 blocks of 128; 8*6400 = 51200 >= 50000
NB = 50                        # node blocks per core
# table rows 1..50000 = nodes 0..49999; row 50001 = zeros (dummy target).
# int16 gather idx = node + 1 - GBASE; dummy idx = +17233 (>= 0, never hits
# the trailing-negative trim). Host swaps an in-block edge so the last
# (trim-order) index of each gather call is >= 0.
TBL_ROWS = 50004
GBASE = 32768
DUMMY_IDX = 50001 - GBASE
SUPER_G = 8                    # groups per gather call (1024 idx)
BF16 = ml_dtypes.bfloat16

_PROG_CACHE = {}


# ---------------- device program ----------------
def _build_program(NG):
    GROUPS = NB * NG
    WINDOWS = GROUPS // 4
    NSUPER = (GROUPS + SUPER_G - 1) // SUPER_G
    IDXW = SUPER_G * 128 // 16       # wrapped idx cols per (full) super
    NJ = (WINDOWS + 3) // 4          # es_w4 column blocks
    ES_CHUNK_J = 3
    OH_CHUNK = 2                     # windows of one-hot per DMA

    nc = bacc.Bacc(num_devices=NCORES, num_swdge_queues=4)
    f32, i16, bf = mybir.dt.float32, mybir.dt.int16, mybir.dt.bfloat16

    tbl = nc.dram_tensor("tbl", [TBL_ROWS, 128], bf, kind="ExternalInput")
    idx_g = nc.dram_tensor("idx_g", [P, NSUPER * IDXW], i16, kind="ExternalInput")
    es4 = nc.dram_tensor("es4", [P, NJ * 512], bf, kind="ExternalInput")
    sh_t = nc.dram_tensor("sh_t", [P, GROUPS * 4], bf, kind="ExternalInput")
    oh_t = nc.dram_tensor("oh_t", [P, GROUPS * 128], bf, kind="ExternalInput")
    w1t = nc.dram_tensor("w1t", [P, 256], bf, kind="ExternalInput")
    w2t = nc.dram_tensor("w2t", [P, 512], bf, kind="ExternalInput")
    nodeout = nc.dram_tensor("nodeout", [NODES_PER_CORE, 32], f32, kind="ExternalOutput")

    AX = mybir.AxisListType.X
    ADD = mybir.AluOpType.add
    MUL_ = mybir.AluOpType.mult
    RELU = mybir.ActivationFunctionType.Relu
    COPY = mybir.ActivationFunctionType.Copy

    with TileContext(nc, trace_sim=_TRACE_SIM) as tc:
        with tc.tile_pool(name="const", bufs=1) as cpool, \
             tc.tile_pool(name="stream", bufs=3) as spool, \
             tc.tile_pool(name="xsp", bufs=4) as xspool, \
             tc.tile_pool(name="work", bufs=3) as wpool, \
             tc.tile_pool(name="psum", bufs=2, space="PSUM") as pp, \
             tc.tile_pool(name="psum1", bufs=1, space="PSUM") as pp1:

            # constants resident in SBUF
            ig_sb = cpool.tile([P, NSUPER * IDXW], i16, name="ig")
            nc.sync.dma_start(ig_sb[:], idx_g[:])
            sh_sb = cpool.tile([P, GROUPS, 4], bf, name="sh")
            nc.sync.dma_start(sh_sb[:], sh_t[:].rearrange("p (g k) -> p g k", k=4))
            w1_sb = cpool.tile([P, 256], bf, name="w1")
            nc.sync.dma_start(w1_sb[:], w1t[:])
            w2_sb = cpool.tile([P, 2, 256], bf, name="w2")
            nc.sync.dma_start(w2_sb[:], w2t[:].rearrange("p (h n) -> p h n", h=2))

            # pre-zero the xs ring: trimmed (padded) gather slots keep stale
            # SBUF content, which must be finite (0 * NaN would poison PSUM)
            for _ in range(4):
                xs_z = xspool.tile([P, SUPER_G, 128], bf, tag="xs")
                nc.vector.memset(xs_z[:], 0.0)

            acc_box = [None]
            xs_sup = None

            def phase_b(st):
                """Products + scatter + block-end for a finished window."""
                g0 = st["g0"]
                shw, xs, w_sb = st["shw"], st["xs"], st["w_sb"]
                ab16, v1s2 = st["ab16"], st["v1s2"]
                oh_sb, ohw = st["oh_sb"], st["ohw"]
                scat = wpool.tile([P, 4, 344], bf, tag="scat")
                # path0/1 (deferred u): scat[:, 0:128] (w,u') = ab16[u']*w01T[w,u']
                nc.vector.tensor_tensor(
                    out=scat[:, :, 0:128].rearrange("p g (w u) -> p g w u", w=8),
                    in0=ab16[:].unsqueeze(2).to_broadcast([P, 4, 8, 16]),
                    in1=w_sb[:, :, 0:128].rearrange("p g (w u) -> p g w u", w=8),
                    op=MUL_,
                )
                # path2 (early u): t2[w] = sum_u s1[u] * w2T[w,u]
                t2p = wpool.tile([P, 4, 8, 8], bf, tag="t2p")
                nc.vector.tensor_tensor(
                    out=t2p[:],
                    in0=xs[:, :, 0:8].unsqueeze(2).to_broadcast([P, 4, 8, 8]),
                    in1=w_sb[:, :, 128:192].rearrange("p g (w u) -> p g w u", w=8),
                    op=MUL_,
                )
                t2 = wpool.tile([P, 4, 8], bf, tag="t2")
                with nc.allow_low_precision("t2 is an 8-term dot, bf16 ok"):
                    nc.vector.tensor_reduce(out=t2[:], in_=t2p[:], axis=AX, op=ADD)
                # scat[:, 128:152] (w, i) = t2[w] * v2[i]
                nc.vector.tensor_tensor(
                    out=scat[:, :, 128:152].rearrange("p g (w i) -> p g w i", w=8),
                    in0=t2[:].unsqueeze(3).to_broadcast([P, 4, 8, 3]),
                    in1=shw[:, :, 1:4].unsqueeze(2).to_broadcast([P, 4, 8, 3]),
                    op=MUL_,
                )
                # path3 (deferred u): scat[:, 152:344] (i,w,u) = v1s2T[i,u]*w3T[w,u]
                # one op per i (DVE ISA allows at most 3 free AP dims)
                for i3 in range(3):
                    nc.vector.tensor_tensor(
                        out=scat[:, :, 152 + 64 * i3 : 216 + 64 * i3].rearrange(
                            "p g (w u) -> p g w u", w=8
                        ),
                        in0=v1s2[:, :, i3, :].unsqueeze(2).to_broadcast([P, 4, 8, 8]),
                        in1=w_sb[:, :, 192:256].rearrange(
                            "p g (w u) -> p g w u", w=8
                        ),
                        op=MUL_,
                    )

                # --- per group: scatter matmul into block accumulator
                for gg in range(4):
                    g = g0 + gg
                    b = g // NG
                    gib = g % NG
                    if gib == 0:
                        acc_new = pp.tile([P, 344], f32, space="PSUM", tag="acc")
                        acc_box[0] = acc_new
                    acc_ps = acc_box[0]
                    nc.tensor.matmul(
                        acc_ps[:],
                        lhsT=oh_sb[:, ohw * 4 + gg, :], rhs=scat[:, gg, :],
                        start=(gib == 0), stop=(gib == NG - 1),
                    )
                    if gib == NG - 1:
                        stage = wpool.tile([P, 32], f32, tag="stage")
                        nc.vector.tensor_reduce(
                            out=stage[:, 0:8],
                            in_=acc_ps[:, 0:128].rearrange(
                                "p (w u) -> p w u", w=8
                            ),
                            axis=AX, op=ADD,
                        )
                        t24 = wpool.tile([P, 24], f32, tag="t24")
                        nc.vector.tensor_reduce(
                            out=t24[:],
                            in_=acc_ps[:, 152:344].rearrange(
                                "p (iw u) -> p iw u", u=8
                            ),
                            axis=AX, op=ADD,
                        )
                        # t24 is (i, w); stage[:, 8:32] wants (w, i)
                        nc.vector.tensor_tensor(
                            out=stage[:, 8:32].rearrange("p (w i) -> p w i", w=8),
                            in0=t24[:].rearrange("p (i w) -> p w i", i=3),
                            in1=acc_ps[:, 128:152].rearrange(
                                "p (w i) -> p w i", w=8
                            ),
                            op=ADD,
                        )
                        nc.sync.dma_start(
                            nodeout[128 * b : 128 * b + 128, :], stage[:]
                        )

            # 3-stage software pipeline:
            #   iter w: MLP1(w)+relu(w)+smalls(w) | MLP2(w-1) | wcopy(w-2)
            #           +prods(w-2)+scatter(w-2)+blockend(w-2)
            st = {}          # per-window deferred state
            for w in range(WINDOWS + 2):
                if w < WINDOWS:
                    c = w % 4
                    j = w // 4
                    g0 = 4 * w

                    # --- stream es chunk (every ES_CHUNK_J col-blocks)
                    if j % ES_CHUNK_J == 0 and c == 0:
                        jw = min(ES_CHUNK_J, NJ - j)
                        es_sb = spool.tile([P, ES_CHUNK_J * 512], bf, tag="es")
                        nc.sync.dma_start(
                            es_sb[:, : jw * 512], es4[:, j * 512 : (j + jw) * 512]
                        )
                    jj = j % ES_CHUNK_J

                    # --- stream one-hot (every OH_CHUNK windows)
                    if w % OH_CHUNK == 0:
                        nwoh = min(OH_CHUNK, WINDOWS - w)
                        oh_sb = spool.tile([P, OH_CHUNK * 4, P], bf, tag="oh")
                        nc.sync.dma_start(
                            oh_sb[:, : nwoh * 4, :],
                            oh_t[:, g0 * 128 : (g0 + nwoh * 4) * 128].rearrange(
                                "p (g n) -> p g n", n=P
                            ),
                        )
                    ohw = w % OH_CHUNK

                    # --- gather (1024 idx = 2 windows per call)
                    if w % 2 == 0:
                        s = w // 2
                        nidx = min(SUPER_G * 128, (GROUPS - s * SUPER_G) * 128)
                        xs_sup = xspool.tile([P, SUPER_G, 128], bf, tag="xs")
                        nc.gpsimd.dma_gather(
                            out_ap=xs_sup[:, : nidx // 128, :],
                            in_ap=tbl[GBASE:, :],
                            idxs_ap=ig_sb[:, s * IDXW : s * IDXW + nidx // 16],
                            num_idxs=nidx, num_idxs_reg=nidx, elem_size=128,
                            queue_num=s % 4,
                        )
                    xs = xs_sup[:, 4 * (w % 2) : 4 * (w % 2) + 4, 0:32]

                    # --- MLP1: h[comp, edge], two 128-comp halves
                    h_ps = pp1.tile([P, 2, 512], f32, space="PSUM", tag="hps")
                    for half in range(2):
                        nc.tensor.matmul(
                            h_ps[:, half, :],
                            lhsT=w1_sb[32 * c : 32 * c + 8,
                                       half * 128 : half * 128 + 128],
                            rhs=es_sb[32 * c : 32 * c + 8,
                                      jj * 512 : jj * 512 + 512],
                            start=True, stop=True,
                            tile_position=(32 * c, 0),
                        )
                    h_sb = wpool.tile([P, 2, 512], bf, tag="hsb")
                    for half in range(2):
                        nc.scalar.activation(
                            out=h_sb[:, half, :], in_=h_ps[:, half, :], func=RELU
                        )

                    # --- per-window DVE smalls (xs/sh only)
                    shw = sh_sb[:, g0 : g0 + 4, :]        # [P, 4, 4]
                    ab16 = wpool.tile([P, 4, 16], bf, tag="ab16")
                    nc.vector.tensor_tensor(
                        out=ab16[:, :, 0:8], in0=xs[:, :, 0:8],
                        in1=shw[:, :, 0:1].to_broadcast([P, 4, 8]), op=MUL_,
                    )
                    pb = wpool.tile([P, 4, 8, 3], bf, tag="pb")
                    nc.vector.tensor_tensor(
                        out=pb[:],
                        in0=xs[:, :, 8:32].rearrange("p g (u i) -> p g u i", u=8),
                        in1=shw[:, :, 1:4].unsqueeze(2).to_broadcast([P, 4, 8, 3]),
                        op=MUL_,
                    )
                    with nc.allow_low_precision("b-dot is 3 terms, bf16 ok"):
                        nc.vector.tensor_reduce(
                            out=ab16[:, :, 8:16], in_=pb[:], axis=AX, op=ADD
                        )
                    v1s2 = wpool.tile([P, 4, 3, 8], bf, tag="v1s2")
                    nc.vector.tensor_tensor(
                        out=v1s2[:],
                        in0=xs[:, :, 8:32].rearrange("p g (u i) -> p g i u", u=8),
                        in1=shw[:, :, 0:1].unsqueeze(2).to_broadcast([P, 4, 3, 8]),
                        op=MUL_,
                    )
                    st[w] = dict(
                        g0=g0, shw=shw, xs=xs, h_sb=h_sb,
                        ab16=ab16, v1s2=v1s2, oh_sb=oh_sb, ohw=ohw,
                    )

                # --- MLP2 for window w-1
                if 0 <= w - 1 < WINDOWS:
                    s1 = st[w - 1]
                    w_ps = pp.tile([P, 4, 256], f32, space="PSUM", tag="wps")
                    for gg in range(4):
                        for half in range(2):
                            nc.tensor.matmul(
                                w_ps[:, gg, :],
                                lhsT=s1["h_sb"][:, half, gg * 128 : gg * 128 + 128],
                                rhs=w2_sb[:, half, :],
                                start=(half == 0), stop=(half == 1),
                            )
                    s1["w_ps"] = w_ps

                # --- wcopy + products + scatter for window w-2
                if w - 2 >= 0:
                    s2 = st[w - 2]
                    w_sb = wpool.tile([P, 4, 256], bf, tag="wsb")
                    nc.scalar.activation(out=w_sb[:], in_=s2["w_ps"][:], func=COPY)
                    s2["w_sb"] = w_sb
                    phase_b(s2)
                    del st[w - 2]
    nc.compile()
    return nc


# ---------------- host-side prep ----------------
def _prep(node_features, edge_src, edge_dst, edge_sh, edge_scalars, fc_w1, fc_w2,
          NG, perm):
    GROUPS = NB * NG
    EPAD = GROUPS * 128
    WINDOWS = GROUPS // 4
    NSUPER = (GROUPS + SUPER_G - 1) // SUPER_G
    IDXW = SUPER_G * 128 // 16
    NJ = (WINDOWS + 3) // 4

    # fold all scalar coefficients into the weights
    w1s = (fc_w1 * (1.0 / math.sqrt(NUM_BASIS))).astype(np.float32)     # [8, 256]
    w2 = (fc_w2 * (SQRT2 / math.sqrt(HIDDEN))).astype(np.float64)       # [256, 256]
    w2 = w2.reshape(HIDDEN, 4, MUL, MUL)
    coef = np.array(
        [A_SCALAR, A_SCALAR * INV_SQRT3, A_VECTOR * INV_SQRT3, A_VECTOR * INV_SQRT3]
    ) * DEG_SCALE
    w2 = w2 * coef[None, :, None, None]
    # device col order, all (w, u) transposed:
    #   [0:128]  w01T: col w*16+u'  (u'<8: path0, u'>=8: path1)
    #   [128:192] w2T: col 128+w*8+u (path2)
    #   [192:256] w3T: col 192+w*8+u (path3)
    w2dev = np.zeros((HIDDEN, 256), np.float64)
    w2dev[:, 0:128] = np.concatenate(
        [w2[:, 0], w2[:, 1]], axis=1  # [256, u8, w8] x2 -> (u', w)
    ).transpose(0, 2, 1).reshape(HIDDEN, 128)          # (w, u')
    w2dev[:, 128:192] = w2[:, 2].transpose(0, 2, 1).reshape(HIDDEN, 64)  # (w, u)
    w2dev[:, 192:256] = w2[:, 3].transpose(0, 2, 1).reshape(HIDDEN, 64)  # (w, u)
    w2dev = w2dev.astype(np.float32)

    w1t = np.zeros((P, 256), np.float32)
    for c in range(4):
        w1t[32 * c : 32 * c + 8] = w1s
    w2t = np.zeros((P, 512), np.float32)
    w2t[:, 0:256] = w2dev[0:128]
    w2t[:, 256:512] = w2dev[128:256]

    tbl = np.zeros((TBL_ROWS, 128), BF16)
    tbl[1 : N_NODES + 1, 0:32] = node_features.astype(BF16)

    src_all = np.asarray(edge_src).astype(np.int64)
    dst_all = np.asarray(edge_dst).astype(np.int64)
    es_all = np.asarray(edge_scalars).astype(np.float32)
    sh_all = np.asarray(edge_sh).astype(np.float32)
    gdst = perm[dst_all]                      # balanced destination slots
    core_of = gdst // NODES_PER_CORE

    nrange = np.arange(P, dtype=np.int64)
    in_maps = []
    for cid in range(NCORES):
        sel = np.nonzero(core_of == cid)[0]
        loc = gdst[sel] - NODES_PER_CORE * cid
        order = np.argsort(loc >> 7, kind="stable")
        sel = sel[order]
        loc = loc[order]
        blk = loc >> 7
        cnt = np.bincount(blk, minlength=NB)
        assert cnt.max() <= NG * 128, (cid, cnt.max())
        start = np.zeros(NB, np.int64)
        start[1:] = np.cumsum(cnt)[:-1]
        rank = np.arange(len(sel)) - start[blk]
        slot = blk * (NG * 128) + rank

        srcv = np.full(EPAD, -1, np.int64)
        srcv[slot] = src_all[sel]
        shv = np.zeros((EPAD, 4), np.float32)
        shv[slot] = sh_all[sel]
        esv = np.zeros((EPAD, 8), np.float32)
        esv[slot] = es_all[sel]
        dlv = np.full(EPAD, -1, np.int64)
        dlv[slot] = loc & 127

        # --- gather indices: row = node+1, idx = node + 1 - GBASE;
        # dummy -> DUMMY_IDX (>= 0, never trimmed). The HW trims trailing
        # NEGATIVE indices per call, so force the last index of each gather
        # call >= 0 by swapping with a same-block edge.
        idxv = np.where(srcv >= 0, srcv + 1 - GBASE, DUMMY_IDX).astype(np.int64)
        BLKE = NG * 128
        SUP = SUPER_G * 128
        for sgi in range(NSUPER):
            jl = min((sgi + 1) * SUP, EPAD) - 1
            if idxv[jl] >= 0:
                continue
            b0 = jl // BLKE
            cand = np.nonzero(idxv[b0 * BLKE : (b0 + 1) * BLKE] >= 0)[0]
            cand = [b0 * BLKE + q for q in cand
                    if (b0 * BLKE + q + 1) % SUP != 0]
            assert cand, "no swap candidate in block"
            q = cand[0]
            for arr in (idxv, srcv, dlv):
                arr[jl], arr[q] = arr[q], arr[jl]
            for arr in (shv, esv):
                tmpq = arr[q].copy()
                arr[q] = arr[jl]
                arr[jl] = tmpq
        idx_pad = np.zeros(NSUPER * SUPER_G * 128, np.int64)
        idx_pad[:EPAD] = idxv
        idx_g = np.tile(
            idx_pad.reshape(-1, 16).T.astype(np.int16), (8, 1)
        )  # [128, NSUPER*IDXW]

        # es4: window w at rows 32*(w%4)+b, cols [ (w//4)*512, +512 )
        es4 = np.zeros((P, NJ * 512), np.float32)
        esw = esv.reshape(WINDOWS, 512, 8)
        for c in range(4):
            wsel = np.arange(c, WINDOWS, 4)
            nw = len(wsel)
            es4[32 * c : 32 * c + 8, : nw * 512] = (
                esw[wsel].transpose(2, 0, 1).reshape(8, nw * 512)
            )

        sh_t = shv.reshape(GROUPS, P, 4).transpose(1, 0, 2).reshape(P, GROUPS * 4)
        # one-hot: oh_t[p, g*128 + n] = (dl[g*128+p] == n)
        dlg = dlv.reshape(GROUPS, P)                      # [g, p]
        oh = (dlg[:, :, None] == nrange[None, None, :])   # [g, p, n]
        oh_t = np.ascontiguousarray(
            oh.transpose(1, 0, 2).reshape(P, GROUPS * 128)
        ).astype(BF16)

        in_maps.append(
            dict(
                tbl=tbl, idx_g=np.ascontiguousarray(idx_g),
                es4=np.ascontiguousarray(es4.astype(BF16)),
                sh_t=np.ascontiguousarray(sh_t.astype(BF16)),
                oh_t=oh_t,
                w1t=w1t.astype(BF16), w2t=w2t.astype(BF16),
            )
        )
    return in_maps


def _plan(edge_dst):
    """Degree-balanced node -> global-slot permutation (snake deal over the
    400 (core, block) bins) and the resulting NG."""
    dst_all = np.asarray(edge_dst).astype(np.int64)
    NBINS = NCORES * NB
    TOT = NBINS * P
    degp = np.zeros(TOT, np.int64)
    degp[:N_NODES] = np.bincount(dst_all, minlength=N_NODES)
    order = np.argsort(-degp, kind="stable")
    i = np.arange(TOT)
    r = i // NBINS                            # deal round = slot within bin
    pos = i % NBINS
    binid = np.where(r % 2 == 0, pos, NBINS - 1 - pos)
    perm = np.empty(TOT, np.int64)
    perm[order] = binid * P + r
    load = np.bincount(binid, weights=degp[order].astype(np.float64),
                       minlength=NBINS).astype(np.int64)
    NG = int(math.ceil(load.max() / 128.0))
    if NG % 2:
        NG += 1
    return max(NG, 2), perm


def kernel(node_features, edge_src, edge_dst, edge_sh, edge_scalars, fc_w1, fc_w2):
    node_features = np.asarray(node_features, dtype=np.float32)
    edge_sh = np.asarray(edge_sh, dtype=np.float32)
    edge_scalars = np.asarray(edge_scalars, dtype=np.float32)
    fc_w1 = np.asarray(fc_w1, dtype=np.float32)
    fc_w2 = np.asarray(fc_w2, dtype=np.float32)

    NG, perm = _plan(edge_dst)
    if NG not in _PROG_CACHE:
        _PROG_CACHE[NG] = _build_program(NG)
    nc = _PROG_CACHE[NG]

    in_maps = _prep(
        node_features, edge_src, edge_dst, edge_sh, edge_scalars, fc_w1, fc_w2,
        NG, perm,
    )
    res = run_bass_kernel_spmd(nc, in_maps, core_ids=list(range(NCORES)))
    out = np.concatenate([res.results[c]["nodeout"] for c in range(NCORES)], axis=0)
    return out[perm[:N_NODES]].astype(np.float32)


# revision 22
# speedup vs baseline: 1.4038x; 1.3753x over previous
"""Trainium2 Bass kernel for nn_Convolution (e3nn-style GNN message passing).

Strategy (8 NeuronCores, SPMD, no collectives):
- Sort edges by destination; core c owns destination nodes [6400c, 6400(c+1)).
- Per core: edges binned into 50 node-blocks (128 nodes each), padded to NG
  groups of 128 edges. Dummy edges gather a zero table row -> contribute 0.
- Gather source features with dma_gather (bf16 table, 256B rows, 1024 idx per
  call = 2 windows) on gpsimd; this is the Q7-descgen-bound roof (~5-7ns/idx).
- Radial MLP layer 1 on PE (bf16, tile_position row-packed K=8), relu on ACT,
  layer 2 on PE (bf16, h stationary); w copied PSUM->SBUF bf16 on ACT.
- TP products on DVE in bf16 with (w,u)-transposed per-edge weights so the
  big products hit 2x/4x DVE modes. Path0/1 and path3 keep the u-contraction
  deferred into the scatter matmul (per-block PSUM accumulation); path2 is
  contracted early (scat width 128+24+192 = 344).
- Scatter via host-baked one-hot (bf16, DMA-streamed) matmuls into PSUM per
  node-block; per-block u-reduction on DVE, then DMA out.
"""

import math
import os
import numpy as np
import ml_dtypes

_TRACE_SIM = bool(int(os.environ.get('K_TRACE_SIM', '0')))

import concourse.bass as bass
import concourse.bacc as bacc
import concourse.mybir as mybir
from concourse.tile import TileContext
from concourse.bass_utils import run_bass_kernel_spmd

# ---------------- problem constants (hardcoded per spec) ----------------
N_NODES, N_EDGES, NUM_BASIS, HIDDEN = 50000, 800000, 8, 256
MUL = 8
INV_SQRT3 = float(1.0 / np.sqrt(3.0))
A_SCALAR = float(np.sqrt(1.0 / 128.0))
A_VECTOR = float(np.sqrt(3.0 / 128.0))
SQRT2 = float(np.sqrt(2.0))
DEG_SCALE = float(1.0 / np.sqrt(N_EDGES / N_NODES))

NCORES = 8
P = 128
NODES_PER_CORE = 6400          # 50 blocks of 128; 8*6400 = 51200 >= 50000
NB = 50                        # node blocks per core
# table rows 1..50000 = nodes 0..49999; row 50001 = zeros (dummy target).
# int16 gather idx = node + 1 - GBASE; dummy idx = +17233 (>= 0, never hits
# the trailing-negative trim). Host swaps an in-block edge so the last
# (trim-order) index of each gather call is >= 0.
TBL_ROWS = 50004
GBASE = 32768
DUMMY_IDX = 50001 - GBASE
SUPER_G = 8                    # groups per gather call (1024 idx)
BF16 = ml_dtypes.bfloat16

_PROG_CACHE = {}


# ---------------- device program ----------------
def _build_program(NG):
    GROUPS = NB * NG
    WINDOWS = GROUPS // 4
    NSUPER = (GROUPS + SUPER_G - 1) // SUPER_G
    IDXW = SUPER_G * 128 // 16       # wrapped idx cols per (full) super
    NJ = (WINDOWS + 3) // 4          # es_w4 column blocks
    ES_CHUNK_J = 3
    OH_CHUNK = 2                     # windows of one-hot per DMA

    nc = bacc.Bacc(num_devices=NCORES, num_swdge_queues=4)
    f32, i16, bf = mybir.dt.float32, mybir.dt.int16, mybir.dt.bfloat16

    tbl = nc.dram_tensor("tbl", [TBL_ROWS, 128], bf, kind="ExternalInput")
    idx_g = nc.dram_tensor("idx_g", [P, NSUPER * IDXW], i16, kind="ExternalInput")
    es4 = nc.dram_tensor("es4", [P, NJ * 512], bf, kind="ExternalInput")
    sh_t = nc.dram_tensor("sh_t", [P, GROUPS * 4], bf, kind="ExternalInput")
    oh_t = nc.dram_tensor("oh_t", [P, GROUPS * 128], bf, kind="ExternalInput")
    w1t = nc.dram_tensor("w1t", [P, 256], bf, kind="ExternalInput")
    w2t = nc.dram_tensor("w2t", [P, 512], bf, kind="ExternalInput")
    nodeout = nc.dram_tensor("nodeout", [NODES_PER_CORE, 32], f32, kind="ExternalOutput")

    AX = mybir.AxisListType.X
    ADD = mybir.AluOpType.add
    MUL_ = mybir.AluOpType.mult
    RELU = mybir.ActivationFunctionType.Relu
    COPY = mybir.ActivationFunctionType.Copy

    with TileContext(nc, trace_sim=_TRACE_SIM) as tc:
        with tc.tile_pool(name="const", bufs=1) as cpool, \
             tc.tile_pool(name="stream", bufs=4) as spool, \
             tc.tile_pool(name="xsp", bufs=6) as xspool, \
             tc.tile_pool(name="work", bufs=3) as wpool, \
             tc.tile_pool(name="psum", bufs=2, space="PSUM") as pp, \
             tc.tile_pool(name="psum1", bufs=1, space="PSUM") as pp1:

            # constants resident in SBUF
            ig_sb = cpool.tile([P, NSUPER * IDXW], i16, name="ig")
            nc.sync.dma_start(ig_sb[:], idx_g[:])
            sh_sb = cpool.tile([P, GROUPS, 4], bf, name="sh")
            nc.sync.dma_start(sh_sb[:], sh_t[:].rearrange("p (g k) -> p g k", k=4))
            w1_sb = cpool.tile([P, 256], bf, name="w1")
            nc.sync.dma_start(w1_sb[:], w1t[:])
            w2_sb = cpool.tile([P, 2, 256], bf, name="w2")
            nc.sync.dma_start(w2_sb[:], w2t[:].rearrange("p (h n) -> p h n", h=2))

            # pre-zero the xs ring: trimmed (padded) gather slots keep stale
            # SBUF content, which must be finite (0 * NaN would poison PSUM)
            for _ in range(6):
                xs_z = xspool.tile([P, SUPER_G, 128], bf, tag="xs")
                nc.vector.memset(xs_z[:], 0.0)

            acc_box = [None]
            xs_sup = None

            def phase_b(st):
                """Products + scatter + block-end for a finished window."""
                g0 = st["g0"]
                shw, xs, w_sb = st["shw"], st["xs"], st["w_sb"]
                ab16, v1s2 = st["ab16"], st["v1s2"]
                oh_sb, ohw = st["oh_sb"], st["ohw"]
                scat = wpool.tile([P, 4, 344], bf, tag="scat")
                # path0/1 (deferred u): scat[:, 0:128] (w,u') = ab16[u']*w01T[w,u']
                nc.vector.tensor_tensor(
                    out=scat[:, :, 0:128].rearrange("p g (w u) -> p g w u", w=8),
                    in0=ab16[:].unsqueeze(2).to_broadcast([P, 4, 8, 16]),
                    in1=w_sb[:, :, 0:128].rearrange("p g (w u) -> p g w u", w=8),
                    op=MUL_,
                )
                # path2 (early u): t2[w] = sum_u s1[u] * w2T[w,u]
                t2p = wpool.tile([P, 4, 8, 8], bf, tag="t2p")
                nc.vector.tensor_tensor(
                    out=t2p[:],
                    in0=xs[:, :, 0:8].unsqueeze(2).to_broadcast([P, 4, 8, 8]),
                    in1=w_sb[:, :, 128:192].rearrange("p g (w u) -> p g w u", w=8),
                    op=MUL_,
                )
                t2 = wpool.tile([P, 4, 8], bf, tag="t2")
                with nc.allow_low_precision("t2 is an 8-term dot, bf16 ok"):
                    nc.vector.tensor_reduce(out=t2[:], in_=t2p[:], axis=AX, op=ADD)
                # scat[:, 128:152] (w, i) = t2[w] * v2[i]
                nc.vector.tensor_tensor(
                    out=scat[:, :, 128:152].rearrange("p g (w i) -> p g w i", w=8),
                    in0=t2[:].unsqueeze(3).to_broadcast([P, 4, 8, 3]),
                    in1=shw[:, :, 1:4].unsqueeze(2).to_broadcast([P, 4, 8, 3]),
                    op=MUL_,
                )
                # path3 (deferred u): scat[:, 152:344] (i,w,u) = v1s2T[i,u]*w3T[w,u]
                # one op per i (DVE ISA allows at most 3 free AP dims)
                for i3 in range(3):
                    nc.vector.tensor_tensor(
                        out=scat[:, :, 152 + 64 * i3 : 216 + 64 * i3].rearrange(
                            "p g (w u) -> p g w u", w=8
                        ),
                        in0=v1s2[:, :, i3, :].unsqueeze(2).to_broadcast([P, 4, 8, 8]),
                        in1=w_sb[:, :, 192:256].rearrange(
                            "p g (w u) -> p g w u", w=8
                        ),
                        op=MUL_,
                    )

                # --- per group: scatter matmul into block accumulator
                for gg in range(4):
                    g = g0 + gg
                    b = g // NG
                    gib = g % NG
                    if gib == 0:
                        acc_new = pp.tile([P, 344], f32, space="PSUM", tag="acc")
                        acc_box[0] = acc_new
                    acc_ps = acc_box[0]
                    nc.tensor.matmul(
                        acc_ps[:],
                        lhsT=oh_sb[:, ohw * 4 + gg, :], rhs=scat[:, gg, :],
                        start=(gib == 0), stop=(gib == NG - 1),
                    )
                    if gib == NG - 1:
                        stage = wpool.tile([P, 32], f32, tag="stage")
                        nc.vector.tensor_reduce(
                            out=stage[:, 0:8],
                            in_=acc_ps[:, 0:128].rearrange(
                                "p (w u) -> p w u", w=8
                            ),
                            axis=AX, op=ADD,
                        )
                        t24 = wpool.tile([P, 24], f32, tag="t24")
                        nc.vector.tensor_reduce(
                            out=t24[:],
                            in_=acc_ps[:, 152:344].rearrange(
                                "p (iw u) -> p iw u", u=8
                            ),
                            axis=AX, op=ADD,
                        )
                        # t24 is (i, w); stage[:, 8:32] wants (w, i)
                        nc.vector.tensor_tensor(
                            out=stage[:, 8:32].rearrange("p (w i) -> p w i", w=8),
                            in0=t24[:].rearrange("p (i w) -> p w i", i=3),
                            in1=acc_ps[:, 128:152].rearrange(
                                "p (w i) -> p w i", w=8
                            ),
                            op=ADD,
                        )
                        nc.sync.dma_start(
                            nodeout[128 * b : 128 * b + 128, :], stage[:]
                        )

            # 3-stage software pipeline:
            #   iter w: MLP1(w)+relu(w)+smalls(w) | MLP2(w-1) | wcopy(w-2)
            #           +prods(w-2)+scatter(w-2)+blockend(w-2)
            st = {}          # per-window deferred state
            for w in range(WINDOWS + 2):
                if w < WINDOWS:
                    c = w % 4
                    j = w // 4
                    g0 = 4 * w

                    # --- stream es chunk (every ES_CHUNK_J col-blocks)
                    if j % ES_CHUNK_J == 0 and c == 0:
                        jw = min(ES_CHUNK_J, NJ - j)
                        es_sb = spool.tile([P, ES_CHUNK_J * 512], bf, tag="es")
                        nc.sync.dma_start(
                            es_sb[:, : jw * 512], es4[:, j * 512 : (j + jw) * 512]
                        )
                    jj = j % ES_CHUNK_J

                    # --- stream one-hot (every OH_CHUNK windows)
                    if w % OH_CHUNK == 0:
                        nwoh = min(OH_CHUNK, WINDOWS - w)
                        oh_sb = spool.tile([P, OH_CHUNK * 4, P], bf, tag="oh")
                        nc.sync.dma_start(
                            oh_sb[:, : nwoh * 4, :],
                            oh_t[:, g0 * 128 : (g0 + nwoh * 4) * 128].rearrange(
                                "p (g n) -> p g n", n=P
                            ),
                        )
                    ohw = w % OH_CHUNK

                    # --- gather (1024 idx = 2 windows per call)
                    if w % 2 == 0:
                        s = w // 2
                        nidx = min(SUPER_G * 128, (GROUPS - s * SUPER_G) * 128)
                        xs_sup = xspool.tile([P, SUPER_G, 128], bf, tag="xs")
                        nc.gpsimd.dma_gather(
                            out_ap=xs_sup[:, : nidx // 128, :],
                            in_ap=tbl[GBASE:, :],
                            idxs_ap=ig_sb[:, s * IDXW : s * IDXW + nidx // 16],
                            num_idxs=nidx, num_idxs_reg=nidx, elem_size=128,
                            queue_num=s % 4,
                        )
                    xs = xs_sup[:, 4 * (w % 2) : 4 * (w % 2) + 4, 0:32]

                    # --- MLP1: h[comp, edge], two 128-comp halves
                    h_ps = pp1.tile([P, 2, 512], f32, space="PSUM", tag="hps")
                    for half in range(2):
                        nc.tensor.matmul(
                            h_ps[:, half, :],
                            lhsT=w1_sb[32 * c : 32 * c + 8,
                                       half * 128 : half * 128 + 128],
                            rhs=es_sb[32 * c : 32 * c + 8,
                                      jj * 512 : jj * 512 + 512],
                            start=True, stop=True,
                            tile_position=(32 * c, 0),
                        )
                    h_sb = wpool.tile([P, 2, 512], bf, tag="hsb")
                    for half in range(2):
                        nc.scalar.activation(
                            out=h_sb[:, half, :], in_=h_ps[:, half, :], func=RELU
                        )

                    # --- per-window DVE smalls (xs/sh only)
                    shw = sh_sb[:, g0 : g0 + 4, :]        # [P, 4, 4]
                    ab16 = wpool.tile([P, 4, 16], bf, tag="ab16")
                    nc.vector.tensor_tensor(
                        out=ab16[:, :, 0:8], in0=xs[:, :, 0:8],
                        in1=shw[:, :, 0:1].to_broadcast([P, 4, 8]), op=MUL_,
                    )
                    pb = wpool.tile([P, 4, 8, 3], bf, tag="pb")
                    nc.vector.tensor_tensor(
                        out=pb[:],
                        in0=xs[:, :, 8:32].rearrange("p g (u i) -> p g u i", u=8),
                        in1=shw[:, :, 1:4].unsqueeze(2).to_broadcast([P, 4, 8, 3]),
                        op=MUL_,
                    )
                    with nc.allow_low_precision("b-dot is 3 terms, bf16 ok"):
                        nc.vector.tensor_reduce(
                            out=ab16[:, :, 8:16], in_=pb[:], axis=AX, op=ADD
                        )
                    v1s2 = wpool.tile([P, 4, 3, 8], bf, tag="v1s2")
                    nc.vector.tensor_tensor(
                        out=v1s2[:],
                        in0=xs[:, :, 8:32].rearrange("p g (u i) -> p g i u", u=8),
                        in1=shw[:, :, 0:1].unsqueeze(2).to_broadcast([P, 4, 3, 8]),
                        op=MUL_,
                    )
                    st[w] = dict(
                        g0=g0, shw=shw, xs=xs, h_sb=h_sb,
                        ab16=ab16, v1s2=v1s2, oh_sb=oh_sb, ohw=ohw,
                    )

                # --- MLP2 for window w-1
                if 0 <= w - 1 < WINDOWS:
                    s1 = st[w - 1]
                    w_ps = pp.tile([P, 4, 256], f32, space="PSUM", tag="wps")
                    for gg in range(4):
                        for half in range(2):
                            nc.tensor.matmul(
                                w_ps[:, gg, :],
                                lhsT=s1["h_sb"][:, half, gg * 128 : gg * 128 + 128],
                                rhs=w2_sb[:, half, :],
                                start=(half == 0), stop=(half == 1),
                            )
                    s1["w_ps"] = w_ps

                # --- wcopy + products + scatter for window w-2
                if w - 2 >= 0:
                    s2 = st[w - 2]
                    w_sb = wpool.tile([P, 4, 256], bf, tag="wsb")
                    nc.scalar.activation(out=w_sb[:], in_=s2["w_ps"][:], func=COPY)
                    s2["w_sb"] = w_sb
                    phase_b(s2)
                    del st[w - 2]
    nc.compile()
    return nc


# ---------------- host-side prep ----------------
def _prep(node_features, edge_src, edge_dst, edge_sh, edge_scalars, fc_w1, fc_w2,
          NG, perm):
    GROUPS = NB * NG
    EPAD = GROUPS * 128
    WINDOWS = GROUPS // 4
    NSUPER = (GROUPS + SUPER_G - 1) // SUPER_G
    IDXW = SUPER_G * 128 // 16
    NJ = (WINDOWS + 3) // 4

    # fold all scalar coefficients into the weights
    w1s = (fc_w1 * (1.0 / math.sqrt(NUM_BASIS))).astype(np.float32)     # [8, 256]
    w2 = (fc_w2 * (SQRT2 / math.sqrt(HIDDEN))).astype(np.float64)       # [256, 256]
    w2 = w2.reshape(HIDDEN, 4, MUL, MUL)
    coef = np.array(
        [A_SCALAR, A_SCALAR * INV_SQRT3, A_VECTOR * INV_SQRT3, A_VECTOR * INV_SQRT3]
    ) * DEG_SCALE
    w2 = w2 * coef[None, :, None, None]
    # device col order, all (w, u) transposed:
    #   [0:128]  w01T: col w*16+u'  (u'<8: path0, u'>=8: path1)
    #   [128:192] w2T: col 128+w*8+u (path2)
    #   [192:256] w3T: col 192+w*8+u (path3)
    w2dev = np.zeros((HIDDEN, 256), np.float64)
    w2dev[:, 0:128] = np.concatenate(
        [w2[:, 0], w2[:, 1]], axis=1  # [256, u8, w8] x2 -> (u', w)
    ).transpose(0, 2, 1).reshape(HIDDEN, 128)          # (w, u')
    w2dev[:, 128:192] = w2[:, 2].transpose(0, 2, 1).reshape(HIDDEN, 64)  # (w, u)
    w2dev[:, 192:256] = w2[:, 3].transpose(0, 2, 1).reshape(HIDDEN, 64)  # (w, u)
    w2dev = w2dev.astype(np.float32)

    w1t = np.zeros((P, 256), np.float32)
    for c in range(4):
        w1t[32 * c : 32 * c + 8] = w1s
    w2t = np.zeros((P, 512), np.float32)
    w2t[:, 0:256] = w2dev[0:128]
    w2t[:, 256:512] = w2dev[128:256]

    tbl = np.zeros((TBL_ROWS, 128), BF16)
    tbl[1 : N_NODES + 1, 0:32] = node_features.astype(BF16)

    src_all = np.asarray(edge_src).astype(np.int64)
    dst_all = np.asarray(edge_dst).astype(np.int64)
    es_all = np.asarray(edge_scalars).astype(np.float32)
    sh_all = np.asarray(edge_sh).astype(np.float32)
    gdst = perm[dst_all]                      # balanced destination slots
    core_of = gdst // NODES_PER_CORE

    nrange = np.arange(P, dtype=np.int64)
    in_maps = []
    for cid in range(NCORES):
        sel = np.nonzero(core_of == cid)[0]
        loc = gdst[sel] - NODES_PER_CORE * cid
        order = np.argsort(loc >> 7, kind="stable")
        sel = sel[order]
        loc = loc[order]
        blk = loc >> 7
        cnt = np.bincount(blk, minlength=NB)
        assert cnt.max() <= NG * 128, (cid, cnt.max())
        start = np.zeros(NB, np.int64)
        start[1:] = np.cumsum(cnt)[:-1]
        rank = np.arange(len(sel)) - start[blk]
        slot = blk * (NG * 128) + rank

        srcv = np.full(EPAD, -1, np.int64)
        srcv[slot] = src_all[sel]
        shv = np.zeros((EPAD, 4), np.float32)
        shv[slot] = sh_all[sel]
        esv = np.zeros((EPAD, 8), np.float32)
        esv[slot] = es_all[sel]
        dlv = np.full(EPAD, -1, np.int64)
        dlv[slot] = loc & 127

        # --- gather indices: row = node+1, idx = node + 1 - GBASE;
        # dummy -> DUMMY_IDX (>= 0, never trimmed). The HW trims trailing
        # NEGATIVE indices per call, so force the last index of each gather
        # call >= 0 by swapping with a same-block edge.
        idxv = np.where(srcv >= 0, srcv + 1 - GBASE, DUMMY_IDX).astype(np.int64)
        BLKE = NG * 128
        SUP = SUPER_G * 128
        for sgi in range(NSUPER):
            jl = min((sgi + 1) * SUP, EPAD) - 1
            if idxv[jl] >= 0:
                continue
            b0 = jl // BLKE
            cand = np.nonzero(idxv[b0 * BLKE : (b0 + 1) * BLKE] >= 0)[0]
            cand = [b0 * BLKE + q for q in cand
                    if (b0 * BLKE + q + 1) % SUP != 0]
            assert cand, "no swap candidate in block"
            q = cand[0]
            for arr in (idxv, srcv, dlv):
                arr[jl], arr[q] = arr[q], arr[jl]
            for arr in (shv, esv):
                tmpq = arr[q].copy()
                arr[q] = arr[jl]
                arr[jl] = tmpq
        idx_pad = np.zeros(NSUPER * SUPER_G * 128, np.int64)
        idx_pad[:EPAD] = idxv
        idx_g = np.tile(
            idx_pad.reshape(-1, 16).T.astype(np.int16), (8, 1)
        )  # [128, NSUPER*IDXW]

        # es4: window w at rows 32*(w%4)+b, cols [ (w//4)*512, +512 )
        es4 = np.zeros((P, NJ * 512), np.float32)
        esw = esv.reshape(WINDOWS, 512, 8)
        for c in range(4):
            wsel = np.arange(c, WINDOWS, 4)
            nw = len(wsel)
            es4[32 * c : 32 * c + 8, : nw * 512] = (
                esw[wsel].transpose(2, 0, 1).reshape(8, nw * 512)
            )

        sh_t = shv.reshape(GROUPS, P, 4).transpose(1, 0, 2).reshape(P, GROUPS * 4)
        # one-hot: oh_t[p, g*128 + n] = (dl[g*128+p] == n)
        dlg = dlv.reshape(GROUPS, P)                      # [g, p]
        oh = (dlg[:, :, None] == nrange[None, None, :])   # [g, p, n]
        oh_t = np.ascontiguousarray(
            oh.transpose(1, 0, 2).reshape(P, GROUPS * 128)
        ).astype(BF16)

        in_maps.append(
            dict(
                tbl=tbl, idx_g=np.ascontiguousarray(idx_g),
                es4=np.ascontiguousarray(es4.astype(BF16)),
                sh_t=np.ascontiguousarray(sh_t.astype(BF16)),
                oh_t=oh_t,
                w1t=w1t.astype(BF16), w2t=w2t.astype(BF16),
            )
        )
    return in_maps


def _plan(edge_dst):
    """Degree-balanced node -> global-slot permutation (snake deal over the
    400 (core, block) bins) and the resulting NG."""
    dst_all = np.asarray(edge_dst).astype(np.int64)
    NBINS = NCORES * NB
    TOT = NBINS * P
    degp = np.zeros(TOT, np.int64)
    degp[:N_NODES] = np.bincount(dst_all, minlength=N_NODES)
    order = np.argsort(-degp, kind="stable")
    i = np.arange(TOT)
    r = i // NBINS                            # deal round = slot within bin
    pos = i % NBINS
    binid = np.where(r % 2 == 0, pos, NBINS - 1 - pos)
    perm = np.empty(TOT, np.int64)
    perm[order] = binid * P + r
    load = np.bincount(binid, weights=degp[order].astype(np.float64),
                       minlength=NBINS).astype(np.int64)
    NG = int(math.ceil(load.max() / 128.0))
    if NG % 2:
        NG += 1
    return max(NG, 2), perm


def kernel(node_features, edge_src, edge_dst, edge_sh, edge_scalars, fc_w1, fc_w2):
    node_features = np.asarray(node_features, dtype=np.float32)
    edge_sh = np.asarray(edge_sh, dtype=np.float32)
    edge_scalars = np.asarray(edge_scalars, dtype=np.float32)
    fc_w1 = np.asarray(fc_w1, dtype=np.float32)
    fc_w2 = np.asarray(fc_w2, dtype=np.float32)

    NG, perm = _plan(edge_dst)
    if NG not in _PROG_CACHE:
        _PROG_CACHE[NG] = _build_program(NG)
    nc = _PROG_CACHE[NG]

    in_maps = _prep(
        node_features, edge_src, edge_dst, edge_sh, edge_scalars, fc_w1, fc_w2,
        NG, perm,
    )
    res = run_bass_kernel_spmd(nc, in_maps, core_ids=list(range(NCORES)))
    out = np.concatenate([res.results[c]["nodeout"] for c in range(NCORES)], axis=0)
    return out[perm[:N_NODES]].astype(np.float32)
